# revision 8
# baseline (speedup 1.0000x reference)
"""KAN layer kernel for 8 Trainium2 NeuronCores.

Math (reference):
    basis[b,i] = sum_h silu(x[b,i]*w1[i%K,h] + b1[i%K,h]) * w2[i%K,h] + b2[i%K]
    out[b,o]   = sum_i basis[b,i] * Wsum[o,i],   Wsum = W.sum(-1)   # [O,I]

Strategy (memory-bound on streaming W; per-core ~21 MB of bf16):
  - Features are permuted so they are sorted by k = i%K.  Each SBUF
    partition then holds NT features of a SINGLE k, so per-feature MLP
    params are per-partition vectors and the basis MLP runs as wide bf16
    2x-mode DVE ops (z = x*w1rep + b1rep per tile; silu / *w2 / h-fold
    tree over groups of 4 tiles), with the two biggest folds on GPSIMD
    and the final +b2 on the scalar engine (ACT bias) -- every engine
    stays far below the DMA roofline.
  - W is cast to bf16 on host (tolerance 2e-2, measured ~4e-3) and
    streamed with plain HWDGE DMAs alternating across BOTH rings
    (sync + scalar queues); consts go first on each ring.  The
    K-reduction rides the PE's PSUM accumulation (170 matmuls), hidden
    under the DMA stream; 8 W buffers decouple DMA from mm latency.
  - Data-parallel over features: core c takes 121 partitions x 17 slots
    of the k-sorted (padded) feature list; partial out[64,1024] summed on
    host.
"""
import numpy as np

B, I, O, K, H = 64, 16384, 1024, 5, 16
NCORES = 8
NT = 17                   # feature slots per partition (= i-tiles per core)
G = 4                     # tiles per basis group (NT = 1 solo + 4 groups)
NG = 4
GP = 193                  # partitions per k-group (ceil(3277/17))
APC = 121                 # active partitions per core (8*121=968 >= 5*193)
NPART = NCORES * APC      # 968 partitions globally
P = 128

TRACE = False             # test.py sets True to capture an NTFF profile
LAST_RESULT = None


def _build():
    from contextlib import ExitStack
    from concourse import bacc, mybir, tile

    f32 = mybir.dt.float32
    bf16 = mybir.dt.bfloat16
    AT = mybir.ActivationFunctionType
    OP = mybir.AluOpType
    nc = bacc.Bacc("TRN2", target_bir_lowering=False, debug=False,
                   num_devices=NCORES)
    Wd = nc.declare_dram_parameter("Wd", [NT, APC, K * O], bf16, isOutput=False)
    xd = nc.declare_dram_parameter("xd", [P, NT * B], bf16, isOutput=False)
    # prd: w1rep [P,H*64] | b1rep [P,H*64] | w2repG [P,H*G*64]
    prd = nc.declare_dram_parameter("prd", [P, (2 * H + H * G) * B], bf16,
                                    isOutput=False)
    fpd = nc.declare_dram_parameter("fpd", [P, 1], f32, isOutput=False)
    out = nc.declare_dram_parameter("out", [B, O], f32, isOutput=True)

    HB = H * B                # 1024
    GW = G * B                # 256: group row width (t,b)
    with tile.TileContext(nc) as tc, ExitStack() as ctx:
        const = ctx.enter_context(tc.tile_pool(name="const", bufs=1))
        wpool = ctx.enter_context(tc.tile_pool(name="w", bufs=10))
        zpool = ctx.enter_context(tc.tile_pool(name="z", bufs=2))
        spool = ctx.enter_context(tc.tile_pool(name="s", bufs=2))
        fpool = ctx.enter_context(tc.tile_pool(name="fold", bufs=2))
        apool = ctx.enter_context(tc.tile_pool(name="acc", bufs=NG + 1))
        opool = ctx.enter_context(tc.tile_pool(name="out", bufs=1))
        psum = ctx.enter_context(tc.tile_pool(name="psum", bufs=1, space="PSUM"))

        # Consts first on each HWDGE ring, ahead of the W stream.
        xsb = const.tile([P, NT * B], bf16)
        nc.sync.dma_start(xsb[:, :], xd[:, :])
        fpsb = const.tile([P, 1], f32)
        nc.sync.dma_start(fpsb[:, :], fpd[:, :])
        prsb = const.tile([P, (2 * H + H * G) * B], bf16)
        nc.scalar.dma_start(prsb[:, :], prd[:, :])
        w1rep = prsb[:, 0:HB]                          # [P,(h,b)]
        b1rep = prsb[:, HB:2 * HB]
        w2g = prsb[:, 2 * HB:2 * HB + H * GW]          # [P,(h,t,b)]
        b2v = fpsb[:, 0:1]

        ps0 = psum.tile([B, 512], f32, tag="ps0")
        ps1 = psum.tile([B, 512], f32, tag="ps1")
        psh = psum.tile([1, B], f32, tag="psh")

        accs = [None] * NT   # per stream-slot: (tile_ap, col0)
        w13 = w1rep.rearrange("p (h b) -> p h b", h=H)

        def heartbeat(src):
            # Tiny matmul reading a just-produced tile: paces PE activity so
            # the HAM clock gate stays at K=8/8 across DMA/dependency waits
            # (a cold PE runs every real matmul at half clock).
            nc.tensor.matmul(psh[:, :], src[:, 0:1], src[:, 0:B],
                             start=True, stop=True)

        def basis_tile(xs, z, s):
            """z,s: [P,HB] tiles; xs: [P,B] slice -> silu(x*w1+b1) in s."""
            xb = xs[:, None, :].to_broadcast((P, H, B))
            nc.vector.tensor_mul(z[:, :].rearrange("p (h b) -> p h b", h=H),
                                 xb, w13)
            nc.vector.tensor_add(z[:, :], z[:, :], b1rep)
            heartbeat(z)
            nc.scalar.activation(s[:, :], z[:, :], AT.Silu)
            heartbeat(s)

        # ---- solo tile (stream slot 0) ----
        z0 = zpool.tile([P, HB], bf16, tag="z0")
        s0 = spool.tile([P, HB], bf16, tag="s0")
        basis_tile(xsb[:, 0:B], z0, s0)
        sw0 = zpool.tile([P, HB], bf16, tag="sw0")
        w2s = w2g.rearrange("p (h j b) -> p h j b", h=H, j=G)[:, :, 0, :]
        nc.vector.tensor_mul(sw0[:, :].rearrange("p (h b) -> p h b", h=H),
                             s0[:, :].rearrange("p (h b) -> p h b", h=H), w2s)
        f80 = fpool.tile([P, 8 * B], bf16, tag="f80")
        nc.vector.tensor_add(f80[:, :], sw0[:, 0:8 * B], sw0[:, 8 * B:16 * B])
        f40 = fpool.tile([P, 4 * B], bf16, tag="f40")
        nc.vector.tensor_add(f40[:, :], f80[:, 0:4 * B], f80[:, 4 * B:8 * B])
        f20 = fpool.tile([P, 2 * B], bf16, tag="f20")
        nc.vector.tensor_add(f20[:, :], f40[:, 0:2 * B], f40[:, 2 * B:4 * B])
        ap0 = fpool.tile([P, B], bf16, tag="ap0")
        nc.vector.tensor_add(ap0[:, :], f20[:, 0:B], f20[:, B:2 * B])
        acc0 = apool.tile([P, B], bf16, tag="acc0")
        nc.scalar.activation(acc0[:, :], ap0[:, :], AT.Identity, bias=b2v)
        accs[0] = (acc0, 0)

        # ---- 4 groups of 4 tiles ----
        for g in range(NG):
            c0 = (1 + g * G) * B
            zs, ss = [], []
            for j in range(G):
                z = zpool.tile([P, HB], bf16, tag=f"zg{j}", name=f"z{g}_{j}")
                s = spool.tile([P, HB], bf16, tag=f"sg{j}", name=f"s{g}_{j}")
                basis_tile(xsb[:, c0 + j * B:c0 + (j + 1) * B], z, s)
                zs.append(z)
                ss.append(s)
            # sw for the whole group in one wide op: s tiles are separate
            # buffers, so multiply per tile into one grouped sw buffer.
            swg = zpool.tile([P, H * GW], bf16, tag="swg", name=f"swg{g}")
            sw4 = swg[:, :].rearrange("p (h j b) -> p h j b", h=H, j=G)
            w24 = w2g.rearrange("p (h j b) -> p h j b", h=H, j=G)
            for j in range(G):
                nc.vector.tensor_mul(
                    sw4[:, :, j, :],
                    ss[j][:, :].rearrange("p (h b) -> p h b", h=H),
                    w24[:, :, j, :])
            f8 = fpool.tile([P, 8 * GW], bf16, tag="f8", name=f"f8_{g}")
            nc.vector.tensor_add(f8[:, :], swg[:, 0:8 * GW],
                                 swg[:, 8 * GW:16 * GW])
            heartbeat(f8)
            f4 = fpool.tile([P, 4 * GW], bf16, tag="f4", name=f"f4_{g}")
            nc.gpsimd.tensor_add(f4[:, :], f8[:, 0:4 * GW], f8[:, 4 * GW:8 * GW])
            f2 = fpool.tile([P, 2 * GW], bf16, tag="f2", name=f"f2_{g}")
            nc.vector.tensor_add(f2[:, :], f4[:, 0:2 * GW], f4[:, 2 * GW:4 * GW])
            apg = fpool.tile([P, GW], bf16, tag="apg", name=f"apg{g}")
            nc.vector.tensor_add(apg[:, :], f2[:, 0:GW], f2[:, GW:2 * GW])
            accg = apool.tile([P, GW], bf16, tag="accg", name=f"accg{g}")
            nc.scalar.activation(accg[:, :], apg[:, :], AT.Identity, bias=b2v)
            for tg in range(G):
                accs[1 + g * G + tg] = (accg, tg * B)

        # ---- stream W on both HWDGE rings, accumulate over (t, k) ----
        for t in range(NT):
            wt = wpool.tile([APC, K * O], bf16, tag="wt", name=f"wt{t}")
            eng = nc.sync if t % 2 == 0 else nc.scalar
            eng.dma_start(wt[:, :], Wd[t])
            heartbeat(wt)
            at, ac = accs[t]
            lhsT = at[0:APC, ac:ac + B]
            for k in range(K):
                st = (t == 0 and k == 0)
                sp = (t == NT - 1 and k == K - 1)
                nc.tensor.matmul(ps0[:, :], lhsT,
                                 wt[:, k * O:k * O + 512], start=st, stop=sp)
                nc.tensor.matmul(ps1[:, :], lhsT,
                                 wt[:, k * O + 512:(k + 1) * O], start=st, stop=sp)

        out_sb = opool.tile([B, O], f32)
        nc.scalar.copy(out_sb[:, 0:512], ps0[:, :])
        nc.vector.tensor_copy(out_sb[:, 512:1024], ps1[:, :])
        nc.sync.dma_start(out[:, :], out_sb[:, :])
    nc.compile()
    return nc


def kernel(x, w1, b1, w2, b2, W):
    global LAST_RESULT
    import ml_dtypes
    from concourse.bass_utils import run_bass_kernel_spmd

    bf16 = ml_dtypes.bfloat16
    x = np.asarray(x, dtype=np.float32)
    W = np.asarray(W, dtype=np.float32)
    w1 = np.asarray(w1, dtype=np.float32)
    b1 = np.asarray(b1, dtype=np.float32)
    w2 = np.asarray(w2, dtype=np.float32)
    b2 = np.asarray(b2, dtype=np.float32)

    # ---- k-sorted feature permutation, padded so every partition holds
    # NT features of a single k ----
    kvec = np.arange(I) % K
    order = np.argsort(kvec, kind="stable")
    counts = [int(np.sum(kvec == k)) for k in range(K)]       # 3277x4, 3276
    plist = np.full(NPART * NT, -1, dtype=np.int64)
    off = 0
    for k in range(K):
        g0 = k * GP * NT
        plist[g0:g0 + counts[k]] = order[off:off + counts[k]]
        off += counts[k]
    feats = plist.reshape(NPART, NT)                          # [968, 17]
    Fidx = np.where(feats < 0, I, feats)                      # pad -> row I
    kpart = np.minimum(np.arange(NPART) // GP, K - 1)         # k per partition

    # ---- host prep ----
    xT = np.concatenate([np.ascontiguousarray(x.T),
                         np.zeros((1, B), np.float32)])       # [I+1, B]
    WT = np.ascontiguousarray(W.reshape(O, I * K).T).reshape(I, K, O)
    WTb = np.concatenate([WT, np.zeros((1, K, O), np.float32)]).astype(bf16)

    w1rep = np.repeat(w1[kpart][:, :, None], B, axis=2).reshape(NPART, H * B)
    b1rep = np.repeat(b1[kpart][:, :, None], B, axis=2).reshape(NPART, H * B)
    w2rep = np.repeat(w2[kpart][:, :, None], G * B, axis=2).reshape(
        NPART, H * G * B)
    b2f = b2[kpart].reshape(NPART, 1)

    in_maps = []
    for c in range(NCORES):
        rows = slice(c * APC, (c + 1) * APC)
        Fc = Fidx[rows]                                       # [121, 17]
        xg = np.zeros((P, NT * B), np.float32)
        xg[:APC] = xT[Fc].reshape(APC, NT * B)
        pr = np.zeros((P, (2 * H + H * G) * B), np.float32)
        pr[:APC, 0:H * B] = w1rep[rows]
        pr[:APC, H * B:2 * H * B] = b1rep[rows]
        pr[:APC, 2 * H * B:] = w2rep[rows]
        fp = np.zeros((P, 1), np.float32)
        fp[:APC] = b2f[rows]
        Wc = np.ascontiguousarray(
            WTb[Fc].transpose(1, 0, 2, 3).reshape(NT, APC, K * O))
        in_maps.append({
            "Wd": Wc,
            "xd": xg.astype(bf16),
            "prd": pr.astype(bf16),
            "fpd": fp,
        })

    nc = _build()
    res = run_bass_kernel_spmd(nc, in_maps, list(range(NCORES)), trace=TRACE)
    LAST_RESULT = res
    out = np.zeros((B, O), dtype=np.float32)
    for c in range(NCORES):
        out += res.results[c]["out"]
    return out
